# revision 17
# baseline (speedup 1.0000x reference)
"""Trainium2 Bass kernel for nn_Attention_77446850281941.

Computes, for dec_hidden [32,1024], enc_outputs [2048,32,1024], W [1,2048], b [1]:
    e[b,s]  = dec_hidden[b]@W[0,:1024] + enc_outputs[s,b,:]@W[0,1024:] + b[0]
    out     = softmax(tanh(e), axis=s)            -> [32, 2048] float32

Sharding: batch (32) is split across 8 NeuronCores (4 rows each); W/b are
replicated. Softmax rows live entirely on one core, so no collectives.

Per-core dataflow: the enc shard [2048, 4, 1024] f32 (32 MB) streams in
s-chunks of 128 (partition = s, free = (b, e); 16 KB contiguous per
partition per DMA).  The weighted e-reduction (sum_e enc*w_enc) is spread
across three engines so it stays under the ~5.6 us/slab DMA stream time
(the previous all-DVE version was VectorE-bound at 1.47 us per column):
 - b=0: GpSimd fused scalar_tensor_tensor (mult + free-axis accumulate).
 - b=1..3: DVE plain tensor_mul into a scratch product (1x f32 rate),
   then ScalarE Identity-activation with accum_out does the reduce.
 - DVE adds the per-b dec_hidden-dot + bias, ScalarE applies tanh then
   exp per chunk.  tanh output is in [-1,1] so exp needs no max shift.
 - Row sums cross partitions via a PE ones-matmul; the final [128, 64]
   tile is PE-transposed so the output DMA writes contiguous 512B rows.
"""

import sys

import numpy as np

for _p in ("/opt/trn_rl_repo",):
    if _p not in sys.path:
        sys.path.insert(0, _p)

import concourse.bacc as bacc
import concourse.tile as tile
from concourse import mybir
from concourse.bass_utils import run_bass_kernel_spmd

F32 = mybir.dt.float32
SRC = 2048          # src_len
BATCH = 32
EH2 = 1024          # 2*enc_hid_dim
DH = 1024           # dec_hid_dim
NCORES = 8
BPC = BATCH // NCORES      # batch rows per core = 4
NCHUNK = SRC // 128        # s-chunks per core = 16
SLAB_BUFS = 6
SPLIT_FIRST = 2            # how many leading slabs get per-b sub-DMAs
PROD_BUFS = 3              # scratch product tiles (mult write -> reduce read)

_NC_CACHE = {}


def build_nc():
    nc = bacc.Bacc("TRN2", target_bir_lowering=False, debug=False)

    enc = nc.dram_tensor("enc", [SRC, BPC, EH2], F32, kind="ExternalInput").ap()
    # w_enc pre-replicated host-side to all 128 partitions
    w1 = nc.dram_tensor("w1", [128, EH2], F32, kind="ExternalInput").ap()
    # dec_bc[p, b] = dec_hidden[b]·w_dec + bias (same for every partition p)
    dbc = nc.dram_tensor("dbc", [128, BPC], F32, kind="ExternalInput").ap()
    # [:, :128] identity; [0:BPC, 128:192] G4 with G4[b, m] = (m//16 == b)
    ident = nc.dram_tensor("ident", [128, 192], F32, kind="ExternalInput").ap()
    out = nc.dram_tensor("out", [BPC * NCHUNK, 128], F32, kind="ExternalOutput").ap()

    MUL = mybir.AluOpType.mult
    ADD = mybir.AluOpType.add
    ACT = mybir.ActivationFunctionType

    with tile.TileContext(nc) as tc:
        with (
            tc.tile_pool(name="consts", bufs=1) as consts,
            tc.tile_pool(name="slabs", bufs=SLAB_BUFS) as slabs,
            tc.tile_pool(name="firsts", bufs=BPC * SPLIT_FIRST) as firsts,
            tc.tile_pool(name="prods", bufs=PROD_BUFS) as prods,
            tc.tile_pool(name="acc", bufs=1) as acc,
            tc.tile_pool(name="small", bufs=1) as small,
            tc.tile_pool(name="psum", bufs=1, space="PSUM") as psum,
        ):
            # w1 (512 KB) leads the sync ring so it lands with the first
            # slab parts; the w3 replica is built on-chip by ScalarE while
            # the first slab DMAs are still in flight.  Small consts ride
            # the scalar HWDGE ring.
            w_sb = consts.tile([128, EH2], F32)
            nc.sync.dma_start(out=w_sb, in_=w1)
            dec_bc = consts.tile([128, BPC], F32)
            nc.scalar.dma_start(out=dec_bc, in_=dbc)
            id_sb = consts.tile([128, 192], F32)
            nc.scalar.dma_start(out=id_sb, in_=ident)
            onec_sb = consts.tile([128, 1], F32)
            nc.gpsimd.memset(onec_sb, 1.0)
            w3_sb = consts.tile([128, BPC - 1, EH2], F32)
            for j in range(BPC - 1):
                nc.scalar.activation(out=w3_sb[:, j, :], in_=w_sb,
                                     func=ACT.Identity)

            # stride-0 dump columns for unused full elementwise results
            dump_g = small.tile([128, 1], F32)
            dump_a = small.tile([128, 1], F32)

            # e_cols[p, b, t] = enc[t*128+p, b, :]·w_enc;  exp_t = exp(tanh(.))
            e_cols = acc.tile([128, BPC, NCHUNK], F32)
            texp = acc.tile([128, BPC, NCHUNK], F32)
            exp_t = acc.tile([128, BPC, NCHUNK], F32)
            for t in range(NCHUNK):
                if t < SPLIT_FIRST:
                    # split the first slab(s) so compute starts after 512 KB
                    parts = []
                    for b_ in range(BPC):
                        sub = firsts.tile([128, EH2], F32, tag="first")
                        nc.sync.dma_start(
                            out=sub, in_=enc[t * 128:(t + 1) * 128, b_, :])
                        parts.append(sub)
                    bslice = lambda b_: parts[b_]
                else:
                    slab = slabs.tile([128, BPC, EH2], F32)
                    nc.sync.dma_start(
                        out=slab, in_=enc[t * 128:(t + 1) * 128, :, :])
                    bslice = lambda b_: slab[:, b_, :]
                # b=1..3: DVE multiply (one wide op), ScalarE accum-reduce.
                # The wide mult goes first so ScalarE unblocks earliest.
                prod = prods.tile([128, BPC - 1, EH2], F32, tag="prod")
                if t < SPLIT_FIRST:
                    for b_ in range(1, BPC):
                        nc.vector.tensor_mul(
                            prod[:, b_ - 1, :], bslice(b_), w_sb)
                else:
                    nc.vector.tensor_mul(prod, slab[:, 1:BPC, :], w3_sb)
                # b=0: DVE fused multiply + free-axis accumulate
                nc.vector.scalar_tensor_tensor(
                    out=dump_g.broadcast_to((128, EH2)),
                    in0=bslice(0), scalar=1.0, in1=w_sb,
                    op0=MUL, op1=MUL, accum_out=e_cols[:, 0, t:t + 1])
                for b_ in range(1, BPC):
                    nc.scalar.activation(
                        out=dump_a.broadcast_to((128, EH2)),
                        in_=prod[:, b_ - 1, :], func=ACT.Identity,
                        accum_out=e_cols[:, b_, t:t + 1])

            # bulk epilogue: tanh(e + dec·w_dec + bias) via the per-partition
            # bias port (dec_bc[p, b] is constant over p), then exp over all
            # 64 (b, t) columns at once -- keeps the slab loop free of the
            # per-chunk DVE<->ScalarE ping-pong.
            for b_ in range(BPC):
                nc.scalar.activation(
                    out=texp[:, b_, :], in_=e_cols[:, b_, :], func=ACT.Tanh,
                    bias=dec_bc[:, b_:b_ + 1], scale=1.0)
            nc.scalar.activation(out=exp_t, in_=texp, func=ACT.Exp)

            # transpose unnormalized exp: [128, (b,t)] -> [(b,t), 128]
            # (runs on PE/ACT in parallel with the denominator chain below)
            p_out = psum.tile([BPC * NCHUNK, 128], F32)
            nc.tensor.transpose(p_out, exp_t[:, :, :], id_sb[:, 0:128])
            out_unn = small.tile([BPC * NCHUNK, 128], F32)
            nc.scalar.activation(out=out_unn, in_=p_out, func=ACT.Identity)

            # denominator: per-b sum over t (DVE) then s (PE), as a column
            sums = small.tile([128, BPC], F32)
            nc.vector.tensor_reduce(
                out=sums, in_=exp_t[:, :, :],
                axis=mybir.AxisListType.X, op=ADD)
            p_tot = psum.tile([BPC, 1], F32)
            nc.tensor.matmul(p_tot, sums, onec_sb)
            tot_sb = small.tile([BPC, 1], F32)
            nc.scalar.activation(out=tot_sb, in_=p_tot, func=ACT.Identity)
            rec_sb = small.tile([BPC, 1], F32)
            nc.vector.reciprocal(rec_sb, tot_sb)
            # broadcast recip_b to the 64 output rows (row r -> b = r//16)
            p_r64 = psum.tile([BPC * NCHUNK, 1], F32)
            nc.tensor.matmul(p_r64, id_sb[0:BPC, 128:192], rec_sb)
            rec64 = small.tile([BPC * NCHUNK, 1], F32)
            nc.scalar.activation(out=rec64, in_=p_r64, func=ACT.Identity)

            # normalize with the per-partition scale port and store
            out_sb = small.tile([BPC * NCHUNK, 128], F32)
            nc.scalar.activation(out=out_sb, in_=out_unn, func=ACT.Identity,
                                 scale=rec64)
            nc.sync.dma_start(out=out, in_=out_sb)

    nc.finalize()
    return nc


def _get_nc():
    if "nc" not in _NC_CACHE:
        _NC_CACHE["nc"] = build_nc()
    return _NC_CACHE["nc"]


def make_in_maps(dec_hidden, enc_outputs, W, b):
    f32 = np.float32
    w_enc = np.asarray(W[0, DH:], dtype=f32)
    w1 = np.ascontiguousarray(np.broadcast_to(w_enc, (128, EH2)).astype(f32))
    ident = np.zeros((128, 192), dtype=f32)
    ident[:, :128] = np.eye(128, dtype=f32)
    for b_ in range(BPC):                   # G4[b, m] = (m // NCHUNK == b)
        ident[b_, 128 + b_ * NCHUNK:128 + (b_ + 1) * NCHUNK] = 1.0
    w_dec = np.asarray(W[0, :DH], dtype=f32)
    bias = np.float32(b[0])
    # dec_contrib[b] = dec_hidden[b]·w_dec + bias (input marshaling, tiny)
    dec_c = (np.asarray(dec_hidden, dtype=f32) @ w_dec + bias).astype(f32)
    in_maps = []
    for i in range(NCORES):
        dbc = np.ascontiguousarray(
            np.broadcast_to(dec_c[i * BPC:(i + 1) * BPC], (128, BPC)))
        in_maps.append({
            "enc": np.ascontiguousarray(
                enc_outputs[:, i * BPC:(i + 1) * BPC, :].astype(f32)),
            "w1": w1,
            "dbc": dbc,
            "ident": ident,
        })
    return in_maps


def assemble_output(results):
    return np.concatenate(
        [r["out"].reshape(BPC, SRC) for r in results], axis=0).astype(np.float32)


def kernel(dec_hidden, enc_outputs, W, b):
    nc = _get_nc()
    in_maps = make_in_maps(dec_hidden, enc_outputs, W, b)
    res = run_bass_kernel_spmd(nc, in_maps, core_ids=list(range(NCORES)))
    return assemble_output(res.results)


# revision 18
# speedup vs baseline: 1.2006x; 1.2006x over previous
"""Trainium2 Bass kernel for nn_Attention_77446850281941.

Computes, for dec_hidden [32,1024], enc_outputs [2048,32,1024], W [1,2048], b [1]:
    e[b,s]  = dec_hidden[b]@W[0,:1024] + enc_outputs[s,b,:]@W[0,1024:] + b[0]
    out     = softmax(tanh(e), axis=s)            -> [32, 2048] float32

Sharding: batch (32) is split across 8 NeuronCores (4 rows each); W/b are
replicated. Softmax rows live entirely on one core, so no collectives.

Per-core dataflow: the enc shard [2048, 4, 1024] f32 (32 MB) streams in
s-chunks of 128 (partition = s, free = (b, e); 16 KB contiguous per
partition per DMA).  The weighted e-reduction (sum_e enc*w_enc) is spread
across three engines so it stays under the ~5.6 us/slab DMA stream time
(the previous all-DVE version was VectorE-bound at 1.47 us per column):
 - b=0: GpSimd fused scalar_tensor_tensor (mult + free-axis accumulate).
 - b=1..3: DVE plain tensor_mul into a scratch product (1x f32 rate),
   then ScalarE Identity-activation with accum_out does the reduce.
 - DVE adds the per-b dec_hidden-dot + bias, ScalarE applies tanh then
   exp per chunk.  tanh output is in [-1,1] so exp needs no max shift.
 - Row sums cross partitions via a PE ones-matmul; the final [128, 64]
   tile is PE-transposed so the output DMA writes contiguous 512B rows.
"""

import sys

import numpy as np

for _p in ("/opt/trn_rl_repo",):
    if _p not in sys.path:
        sys.path.insert(0, _p)

import concourse.bacc as bacc
import concourse.tile as tile
from concourse import mybir
from concourse.bass_utils import run_bass_kernel_spmd

F32 = mybir.dt.float32
SRC = 2048          # src_len
BATCH = 32
EH2 = 1024          # 2*enc_hid_dim
DH = 1024           # dec_hid_dim
NCORES = 8
BPC = BATCH // NCORES      # batch rows per core = 4
NCHUNK = SRC // 128        # s-chunks per core = 16
SLAB_BUFS = 6
SPLIT_FIRST = 2            # how many leading slabs get per-b sub-DMAs
PROD_BUFS = 3              # scratch product tiles (mult write -> reduce read)

_NC_CACHE = {}


def build_nc():
    nc = bacc.Bacc("TRN2", target_bir_lowering=False, debug=False)

    enc = nc.dram_tensor("enc", [SRC, BPC, EH2], F32, kind="ExternalInput").ap()
    # w_enc pre-replicated host-side to all 128 partitions
    w1 = nc.dram_tensor("w1", [128, EH2], F32, kind="ExternalInput").ap()
    # dec_bc[p, b] = dec_hidden[b]·w_dec + bias (same for every partition p)
    dbc = nc.dram_tensor("dbc", [128, BPC], F32, kind="ExternalInput").ap()
    # [:, :128] identity; [0:BPC, 128:192] G4 with G4[b, m] = (m//16 == b)
    ident = nc.dram_tensor("ident", [128, 192], F32, kind="ExternalInput").ap()
    out = nc.dram_tensor("out", [BPC * NCHUNK, 128], F32, kind="ExternalOutput").ap()

    MUL = mybir.AluOpType.mult
    ADD = mybir.AluOpType.add
    ACT = mybir.ActivationFunctionType

    with tile.TileContext(nc) as tc:
        with (
            tc.tile_pool(name="consts", bufs=1) as consts,
            tc.tile_pool(name="slabs", bufs=SLAB_BUFS) as slabs,
            tc.tile_pool(name="firsts", bufs=BPC * SPLIT_FIRST) as firsts,
            tc.tile_pool(name="prods", bufs=PROD_BUFS) as prods,
            tc.tile_pool(name="acc", bufs=1) as acc,
            tc.tile_pool(name="small", bufs=1) as small,
            tc.tile_pool(name="psum", bufs=1, space="PSUM") as psum,
        ):
            # w1 (512 KB) leads the sync ring so it lands with the first
            # slab parts; the w3 replica is built on-chip by ScalarE while
            # the first slab DMAs are still in flight.  Small consts ride
            # the scalar HWDGE ring.
            w3_sb = consts.tile([128, BPC - 1, EH2], F32)
            nc.sync.dma_start(out=w3_sb[:, 0, :], in_=w1)
            w_sb = w3_sb[:, 0, :]
            dec_bc = consts.tile([128, BPC], F32)
            nc.scalar.dma_start(out=dec_bc, in_=dbc)
            id_sb = consts.tile([128, 192], F32)
            nc.scalar.dma_start(out=id_sb, in_=ident)
            onec_sb = consts.tile([128, 1], F32)
            nc.gpsimd.memset(onec_sb, 1.0)
            for j in range(1, BPC - 1):
                nc.scalar.activation(out=w3_sb[:, j, :], in_=w_sb,
                                     func=ACT.Identity)

            # stride-0 dump columns for unused full elementwise results
            dump_g = small.tile([128, 1], F32)
            dump_a = small.tile([128, 1], F32)

            # e_cols[p, b, t] = enc[t*128+p, b, :]·w_enc;  exp_t = exp(tanh(.))
            e_cols = acc.tile([128, BPC, NCHUNK], F32)
            texp = acc.tile([128, BPC, NCHUNK], F32)
            exp_t = acc.tile([128, BPC, NCHUNK], F32)
            for t in range(NCHUNK):
                if t < SPLIT_FIRST:
                    # split the first slab(s) so compute starts after 512 KB
                    parts = []
                    for b_ in range(BPC):
                        sub = firsts.tile([128, EH2], F32, tag="first")
                        nc.sync.dma_start(
                            out=sub, in_=enc[t * 128:(t + 1) * 128, b_, :])
                        parts.append(sub)
                    bslice = lambda b_: parts[b_]
                else:
                    slab = slabs.tile([128, BPC, EH2], F32)
                    nc.sync.dma_start(
                        out=slab, in_=enc[t * 128:(t + 1) * 128, :, :])
                    bslice = lambda b_: slab[:, b_, :]
                # b=1..3: DVE multiply (one wide op), ScalarE accum-reduce.
                # The wide mult goes first so ScalarE unblocks earliest.
                prod = prods.tile([128, BPC - 1, EH2], F32, tag="prod")
                if t < SPLIT_FIRST:
                    for b_ in range(1, BPC):
                        nc.vector.tensor_mul(
                            prod[:, b_ - 1, :], bslice(b_), w_sb)
                else:
                    nc.vector.tensor_mul(prod, slab[:, 1:BPC, :], w3_sb)
                # b=0: DVE fused multiply + free-axis accumulate
                nc.vector.scalar_tensor_tensor(
                    out=dump_g.broadcast_to((128, EH2)),
                    in0=bslice(0), scalar=1.0, in1=w_sb,
                    op0=MUL, op1=MUL, accum_out=e_cols[:, 0, t:t + 1])
                for b_ in range(1, BPC):
                    nc.scalar.activation(
                        out=dump_a.broadcast_to((128, EH2)),
                        in_=prod[:, b_ - 1, :], func=ACT.Identity,
                        accum_out=e_cols[:, b_, t:t + 1])

            # bulk epilogue: tanh(e + dec·w_dec + bias) via the per-partition
            # bias port (dec_bc[p, b] is constant over p), then exp over all
            # 64 (b, t) columns at once -- keeps the slab loop free of the
            # per-chunk DVE<->ScalarE ping-pong.
            for b_ in range(BPC):
                nc.scalar.activation(
                    out=texp[:, b_, :], in_=e_cols[:, b_, :], func=ACT.Tanh,
                    bias=dec_bc[:, b_:b_ + 1], scale=1.0)
            nc.scalar.activation(out=exp_t, in_=texp, func=ACT.Exp)

            # transpose unnormalized exp: [128, (b,t)] -> [(b,t), 128]
            # (runs on PE/ACT in parallel with the denominator chain below)
            p_out = psum.tile([BPC * NCHUNK, 128], F32)
            nc.tensor.transpose(p_out, exp_t[:, :, :], id_sb[:, 0:128])
            out_unn = small.tile([BPC * NCHUNK, 128], F32)
            nc.scalar.activation(out=out_unn, in_=p_out, func=ACT.Identity)

            # denominator: per-b sum over t (DVE) then s (PE), as a column
            sums = small.tile([128, BPC], F32)
            nc.vector.tensor_reduce(
                out=sums, in_=exp_t[:, :, :],
                axis=mybir.AxisListType.X, op=ADD)
            p_tot = psum.tile([BPC, 1], F32)
            nc.tensor.matmul(p_tot, sums, onec_sb)
            tot_sb = small.tile([BPC, 1], F32)
            nc.scalar.activation(out=tot_sb, in_=p_tot, func=ACT.Identity)
            rec_sb = small.tile([BPC, 1], F32)
            nc.vector.reciprocal(rec_sb, tot_sb)
            # broadcast recip_b to the 64 output rows (row r -> b = r//16)
            p_r64 = psum.tile([BPC * NCHUNK, 1], F32)
            nc.tensor.matmul(p_r64, id_sb[0:BPC, 128:192], rec_sb)
            rec64 = small.tile([BPC * NCHUNK, 1], F32)
            nc.scalar.activation(out=rec64, in_=p_r64, func=ACT.Identity)

            # normalize with the per-partition scale port and store
            out_sb = small.tile([BPC * NCHUNK, 128], F32)
            nc.scalar.activation(out=out_sb, in_=out_unn, func=ACT.Identity,
                                 scale=rec64)
            nc.sync.dma_start(out=out, in_=out_sb)

    nc.finalize()
    return nc


def _get_nc():
    if "nc" not in _NC_CACHE:
        _NC_CACHE["nc"] = build_nc()
    return _NC_CACHE["nc"]


def make_in_maps(dec_hidden, enc_outputs, W, b):
    f32 = np.float32
    w_enc = np.asarray(W[0, DH:], dtype=f32)
    w1 = np.ascontiguousarray(np.broadcast_to(w_enc, (128, EH2)).astype(f32))
    ident = np.zeros((128, 192), dtype=f32)
    ident[:, :128] = np.eye(128, dtype=f32)
    for b_ in range(BPC):                   # G4[b, m] = (m // NCHUNK == b)
        ident[b_, 128 + b_ * NCHUNK:128 + (b_ + 1) * NCHUNK] = 1.0
    w_dec = np.asarray(W[0, :DH], dtype=f32)
    bias = np.float32(b[0])
    # dec_contrib[b] = dec_hidden[b]·w_dec + bias (input marshaling, tiny)
    dec_c = (np.asarray(dec_hidden, dtype=f32) @ w_dec + bias).astype(f32)
    in_maps = []
    for i in range(NCORES):
        dbc = np.ascontiguousarray(
            np.broadcast_to(dec_c[i * BPC:(i + 1) * BPC], (128, BPC)))
        in_maps.append({
            "enc": np.ascontiguousarray(
                enc_outputs[:, i * BPC:(i + 1) * BPC, :].astype(f32)),
            "w1": w1,
            "dbc": dbc,
            "ident": ident,
        })
    return in_maps


def assemble_output(results):
    return np.concatenate(
        [r["out"].reshape(BPC, SRC) for r in results], axis=0).astype(np.float32)


def kernel(dec_hidden, enc_outputs, W, b):
    nc = _get_nc()
    in_maps = make_in_maps(dec_hidden, enc_outputs, W, b)
    res = run_bass_kernel_spmd(nc, in_maps, core_ids=list(range(NCORES)))
    return assemble_output(res.results)


# revision 20
# speedup vs baseline: 1.9303x; 1.6077x over previous
"""Trainium2 Bass kernel for nn_Attention_77446850281941.

Computes, for dec_hidden [32,1024], enc_outputs [2048,32,1024], W [1,2048], b [1]:
    e[b,s]  = dec_hidden[b]@W[0,:1024] + enc_outputs[s,b,:]@W[0,1024:] + b[0]
    out     = softmax(tanh(e), axis=s)            -> [32, 2048] float32

Sharding: batch (32) is split across 8 NeuronCores (4 rows each); W/b are
replicated.  Softmax rows live entirely on one core, so no collectives.

The dominant cost is streaming enc (256 MB f32 over the chip).  Host-side
marshaling casts enc to fp16 (tolerance is 2e-2; fp16 + f32 PSUM
accumulation lands ~1e-4) and pre-transposes each core's shard so the
contraction axis e sits on SBUF partitions:

    enc_t[sb, p, c, s, b] = enc[sb*256+s, b, c*128+p]   (fp16)

Per slab sb (2.1 MB, 16 KB/partition contiguous -> full DMA rate), the
TensorEngine does the whole weighted reduction as a matvec, consuming
128 elem/cycle (~2.5x the f32 DVE path, fully hidden under DMA):

    p_e[1, s, b] += w_cols[:, c].T @ slab[:, c, s, b]      (8 chunk matmuls)
    p_e          += ones.T @ dec_pattern                   (K=1 bias matmul)

ScalarE applies tanh then exp ([1, 1024] rows); DVE accumulates per-b
partial softmax denominators.  The epilogue scatters the unnormalized exp
row (partition 0, 32 KB) across 128 partitions with one SBUF->SBUF DMA,
multiplies by the broadcast reciprocal row sums, and DMAs out 32 KB whose
(s, b) decode happens in the host-side unshard.
"""

import sys

import numpy as np

for _p in ("/opt/trn_rl_repo",):
    if _p not in sys.path:
        sys.path.insert(0, _p)

import concourse.bacc as bacc
import concourse.tile as tile
from concourse import mybir
from concourse.bass_utils import run_bass_kernel_spmd

F32 = mybir.dt.float32
F16 = mybir.dt.float16
SRC = 2048          # src_len
BATCH = 32
EH2 = 1024          # 2*enc_hid_dim
DH = 1024           # dec_hid_dim
NCORES = 8
BPC = BATCH // NCORES      # batch rows per core = 4
NCHUNK = EH2 // 128        # e-chunks = 8
SBLK = 256                 # s-values per slab
NSLAB = SRC // SBLK        # slabs per core = 8
SLAB_BUFS = 5

_NC_CACHE = {}


def build_nc():
    nc = bacc.Bacc("TRN2", target_bir_lowering=False, debug=False)

    enc = nc.dram_tensor("enc", [NSLAB, 128, NCHUNK, 2, SBLK // 2, BPC], F16,
                         kind="ExternalInput").ap()
    wc = nc.dram_tensor("wc", [128, NCHUNK], F16, kind="ExternalInput").ap()
    # dec_pattern[0, s, b] = dec_hidden[b]·w_dec + bias  (tiny, host-packed)
    dpat = nc.dram_tensor("dpat", [1, 2, SBLK // 2, BPC], F16,
                          kind="ExternalInput").ap()
    out = nc.dram_tensor("out", [128, SRC * BPC // 128], F32,
                         kind="ExternalOutput").ap()

    ADD = mybir.AluOpType.add
    MUL = mybir.AluOpType.mult
    ACT = mybir.ActivationFunctionType

    with tile.TileContext(nc) as tc:
        with (
            tc.tile_pool(name="consts", bufs=1) as consts,
            tc.tile_pool(name="slabs", bufs=SLAB_BUFS) as slabs,
            tc.tile_pool(name="rows", bufs=2) as rows,
            tc.tile_pool(name="small", bufs=1) as small,
            tc.tile_pool(name="psum", bufs=2, space="PSUM") as psum,
            tc.tile_pool(name="psum1", bufs=1, space="PSUM") as psum1,
        ):
            w_sb = consts.tile([128, NCHUNK], F16)
            nc.sync.dma_start(out=w_sb, in_=wc)
            dpat_sb = consts.tile([1, 2, SBLK // 2, BPC], F16)
            nc.scalar.dma_start(out=dpat_sb, in_=dpat)
            ones11 = consts.tile([1, 1], F16)
            nc.gpsimd.memset(ones11, 1.0)
            ones128 = consts.tile([1, 128], F32)
            nc.gpsimd.memset(ones128, 1.0)

            # unnormalized exp rows, one 4 KB row per slab (partition 0)
            exp_all = small.tile([1, NSLAB, 2, SBLK // 2, BPC], F32)
            parts = small.tile([1, NSLAB, BPC], F32)

            for sb in range(NSLAB):
                slab = slabs.tile([128, NCHUNK, 2, SBLK // 2, BPC], F16)
                nc.sync.dma_start(out=slab, in_=enc[sb])
                # e row: 8 chunk matvecs + K=1 bias matmul per PSUM-bank
                # half (a matmul output cannot cross a 2 KB PSUM bank)
                p_e = psum.tile([1, 2, SBLK // 2, BPC], F32)
                for h in range(2):
                    for c in range(NCHUNK):
                        nc.tensor.matmul(
                            p_e[:, h, :, :], w_sb[:, c:c + 1],
                            slab[:, c, h, :, :], start=(c == 0), stop=False)
                    nc.tensor.matmul(
                        p_e[:, h, :, :], ones11, dpat_sb[:, h, :, :],
                        start=False, stop=True)
                trow = rows.tile([1, 2, SBLK // 2, BPC], F32, tag="trow")
                nc.scalar.activation(out=trow, in_=p_e, func=ACT.Tanh)
                nc.scalar.activation(
                    out=exp_all[:, sb, :, :, :], in_=trow, func=ACT.Exp)
                # per-b partial denominators for this slab
                for b_ in range(BPC):
                    nc.vector.tensor_reduce(
                        out=parts[:, sb, b_:b_ + 1],
                        in_=exp_all[:, sb, :, :, b_],
                        axis=mybir.AxisListType.XY, op=ADD)

            # denominators -> reciprocals -> broadcast to all partitions
            tot = small.tile([1, BPC], F32)
            for b_ in range(BPC):
                nc.vector.tensor_reduce(
                    out=tot[:, b_:b_ + 1], in_=parts[:, :, b_],
                    axis=mybir.AxisListType.X, op=ADD)
            rec = small.tile([1, BPC], F32)
            nc.vector.reciprocal(rec, tot)
            p_recb = psum1.tile([128, BPC], F32)
            nc.tensor.matmul(p_recb, ones128, rec)
            recb = small.tile([128, 1, BPC], F32)
            nc.scalar.activation(out=recb[:, 0, :], in_=p_recb,
                                 func=ACT.Identity)

            # scatter the 32 KB exp row across 128 partitions (64 elem each),
            # normalize with the per-(partition,b) reciprocal, store
            spread = small.tile([128, SRC * BPC // (128 * BPC), BPC], F32)
            nc.sync.dma_start(out=spread, in_=exp_all)
            out_sb = small.tile([128, SRC * BPC // (128 * BPC), BPC], F32)
            nc.vector.tensor_tensor(
                out=out_sb, in0=spread,
                in1=recb.broadcast_to((128, SRC * BPC // (128 * BPC), BPC)),
                op=MUL)
            nc.sync.dma_start(out=out, in_=out_sb)

    nc.finalize()
    return nc


def _get_nc():
    if "nc" not in _NC_CACHE:
        _NC_CACHE["nc"] = build_nc()
    return _NC_CACHE["nc"]


def make_in_maps(dec_hidden, enc_outputs, W, b):
    f32, f16 = np.float32, np.float16
    w_enc = np.asarray(W[0, DH:], dtype=f32)
    wc = np.ascontiguousarray(w_enc.reshape(NCHUNK, 128).T.astype(f16))
    w_dec = np.asarray(W[0, :DH], dtype=f32)
    bias = np.float32(b[0])
    dec_c = (np.asarray(dec_hidden, dtype=f32) @ w_dec + bias).astype(f32)
    enc_f = np.asarray(enc_outputs, dtype=f32)
    in_maps = []
    for i in range(NCORES):
        sl = slice(i * BPC, (i + 1) * BPC)
        # [2048, 4, 1024] -> [sb, h, s, b, c, p] -> [sb, p, c, h, s, b]
        enc_t = (enc_f[:, sl, :]
                 .reshape(NSLAB, 2, SBLK // 2, BPC, NCHUNK, 128)
                 .transpose(0, 5, 4, 1, 2, 3)
                 .astype(f16))
        dpat = np.broadcast_to(dec_c[sl].astype(f16), (1, 2, SBLK // 2, BPC))
        in_maps.append({
            "enc": np.ascontiguousarray(enc_t),
            "wc": wc,
            "dpat": np.ascontiguousarray(dpat),
        })
    return in_maps


def assemble_output(results):
    # out[m, j] = flat[m*64 + j] with flat = (sb, s, b) of the exp row
    outs = []
    for r in results:
        flat = r["out"].reshape(SRC * BPC)
        outs.append(flat.reshape(SRC, BPC).T)  # (s_g, b) -> [b, s]
    return np.ascontiguousarray(np.concatenate(outs, axis=0)).astype(np.float32)


def kernel(dec_hidden, enc_outputs, W, b):
    nc = _get_nc()
    in_maps = make_in_maps(dec_hidden, enc_outputs, W, b)
    res = run_bass_kernel_spmd(nc, in_maps, core_ids=list(range(NCORES)))
    return assemble_output(res.results)
